# revision 29
# baseline (speedup 1.0000x reference)
"""Trainium2 Bass kernel for one BLT transformer layer (B=2, S=2048, D=2048,
H=16, KVH=4, HD=128, I=8192, fp32 I/O).

Sharding: sequence-parallel over 8 cores with one K/V AllGather. Core c
handles batch b=c//4 and 512 query tokens chosen as 8 causally-balanced
64-token blocks. K/V projection work is deduplicated: each core computes
K/V (+RMSNorm rstd) only for its contiguous 512-token kv slice, then the
4 cores of a batch AllGather the rope'd/normalized K (fp16) and V (bf16)
through HBM bounce buffers while the Q projections keep the PE busy.
Attention (causal-skip), o-proj, and the full MLP for the core's 512
query tokens are unchanged from the no-collective version.

On-chip layout is feature-major [feature, token] throughout, so every
matmul contracts along the partition dim with no on-chip transposes.
Matmuls run in fp16 (fp32 PSUM accumulation); softmax/norm math in fp32.
The rstd broadcasts ride a K=1 fp16 matmul (fp32 rows cost 4 cycles/col
on the PE); the per-head softmax normalizer is broadcast by a gpsimd
ucode op instead, keeping the wide-range 1/den in fp32 (it spans e^-17..
e^0 and underflows fp16) and freeing a psum bank per head. The x stream
rides the SP DGE rings while weight loads and the gather readback use the
Activation-engine rings, so the AllGather path is never queued behind the
weight stream. RoPE's interleaved pairs become contiguous halves via a
host-side even/odd permutation of the wq/wk rows. Softmax skips the
max-subtraction (exp fits in bf16) and applies the causal mask as a
binary multiply on exp(scores); with the ascending-extent block order
only one 64-query block per kv chunk can be partial/invalid, so the mask
multiply touches just 64 columns per chunk.
"""

import os
from contextlib import ExitStack

import ml_dtypes
import numpy as np

import concourse.bacc as bacc
import concourse.mybir as mybir
import concourse.tile as tile
from concourse.bass_utils import run_bass_kernel_spmd
from concourse.masks import make_identity

F8 = mybir.dt.float8e4
F16 = mybir.dt.float16
BF16 = mybir.dt.bfloat16
F32 = mybir.dt.float32
AF = mybir.ActivationFunctionType
OP = mybir.AluOpType
DR = mybir.MatmulPerfMode.DoubleRow

P = 128
EPS = 1e-6
NEG_THRESH = -0.5  # additive mask values are 0.0 or -1e9

FULL_CFG = dict(D=2048, TKV=2048, TQ=512, H=16, KVH=4, I=8192)

# Causal-skip attention. Each core handles 8 query blocks of 64 tokens from
# one batch, chosen so the multiset of causal extents is balanced across
# cores (blocks {j, 7-j, 8+j, 15-j, 16+j, 23-j, 24+j, 31-j} for j = core%4)
# and laid out in the 512-query space in ascending-extent order. A single
# SPMD program must have one shape, so the static per-chunk active width is
# the max extent over cores: kv chunk kc only needs query columns
# [64*(kc//2), 512). Scores/AV/denominator instruction count stays at the
# dense kernel's 16 per head, but columns drop to 56.25%; per-core validity
# inside the active width is data-driven via the host-built causal mask.
BLK = 64
N_BLK = 32                               # 64-token blocks per batch
OFF = [BLK * (kc // 2) for kc in range(16)]   # first active column per chunk

REPLICA_GROUPS = [[0, 1, 2, 3], [4, 5, 6, 7]]

LAST_EXEC_NS = None


# --------------------------------------------------------------------------
# kernel body (built once per process)
# --------------------------------------------------------------------------

def build_nc(cfg, debug=False):
    D, TKV, TQ, H, KVH, I = (cfg[k] for k in ("D", "TKV", "TQ", "H", "KVH", "I"))
    DC = D // P          # d-model chunks
    KC = TKV // P        # kv-token chunks (full batch)
    IT = I // P          # intermediate tiles
    DV = KVH * P         # v width
    TL = TQ              # local kv-slice length (512 tokens per core)
    KCL = TL // P        # local kv chunks
    assert TQ <= 512 and DV <= 512

    nc = bacc.Bacc("TRN2", target_bir_lowering=False, debug=debug)

    t = {}
    t["xT"] = nc.dram_tensor("xT", [D, TL], F16, kind="ExternalInput")
    t["xq"] = nc.dram_tensor("xq", [D, TQ], F32, kind="ExternalInput")
    t["xq16"] = nc.dram_tensor("xq16", [D, TQ], F16, kind="ExternalInput")
    t["cos_q"] = nc.dram_tensor("cos_q", [64, TQ], F32, kind="ExternalInput")
    t["sin_q"] = nc.dram_tensor("sin_q", [64, TQ], F32, kind="ExternalInput")
    t["cos_k"] = nc.dram_tensor("cos_k", [64, TL], F32, kind="ExternalInput")
    t["sin_k"] = nc.dram_tensor("sin_k", [64, TL], F32, kind="ExternalInput")
    t["maskb"] = nc.dram_tensor("maskb", [TKV // P, P, BLK], BF16,
                                kind="ExternalInput")
    t["wq_t"] = nc.dram_tensor("wq_t", [H, P, DC, P], F16, kind="ExternalInput")
    t["wk_t"] = nc.dram_tensor("wk_t", [KVH, P, DC, P], F16, kind="ExternalInput")
    t["wv_r"] = nc.dram_tensor("wv_r", [DC, P, DV], F16, kind="ExternalInput")
    t["wo_t"] = nc.dram_tensor("wo_t", [DC, P, H, P], F16, kind="ExternalInput")
    t["wg_t"] = nc.dram_tensor("wg_t", [IT, P, DC, P], F16, kind="ExternalInput")
    t["wu_t"] = nc.dram_tensor("wu_t", [IT, P, DC, P], F16, kind="ExternalInput")
    t["wd_t"] = nc.dram_tensor("wd_t", [DC, P, IT, P], F16, kind="ExternalInput")
    t["outT"] = nc.dram_tensor("outT", [D, TQ], F32, kind="ExternalOutput")
    # HBM bounce buffers for the K/V AllGather within each 4-core group.
    # One combined collective: slots 0..KVH-1 hold K (fp16), slots
    # KVH..KVH+KCL-1 hold V (bf16 bitcast to fp16 bytes).
    t["kvb_in"] = nc.dram_tensor("kvb_in", [P, KVH + KCL, TL], F16,
                                 kind="Internal")
    t["kvb_out"] = nc.dram_tensor("kvb_out", [4, P, KVH + KCL, TL], F16,
                                  kind="Internal")

    with tile.TileContext(nc) as tc:
        _body(nc, tc, t, D, TKV, TQ, H, KVH, I, DC, KC, IT, DV, TL, KCL)
    nc.compile()
    return nc


def _body(nc, tc, t, D, TKV, TQ, H, KVH, I, DC, KC, IT, DV, TL, KCL):
    with ExitStack() as ctx:
        # global pools: small constants + one PSUM pool budgeted to 8 banks
        # (proj 3 + scores 2 + av 2 + small 1).
        misc = ctx.enter_context(tc.tile_pool(name="misc", bufs=1, side="right"))
        psum = ctx.enter_context(tc.tile_pool(name="psum", bufs=1, space="PSUM"))

        ones16 = misc.tile([P, 1], F16, tag="ones16")
        nc.vector.memset(ones16[:], 1.0)
        ones16r = misc.tile([1, P], F16, tag="ones16r")
        nc.vector.memset(ones16r[:], 1.0)
        ones_bfr = misc.tile([1, P], BF16, tag="ones_bfr")
        nc.vector.memset(ones_bfr[:], 1.0)
        ones_bf = misc.tile([P, 1], BF16, tag="ones_bf")
        nc.vector.memset(ones_bf[:], 1.0)

        def recip(out_ap, in_ap):
            sc = misc.tile([1, 512], F32, tag="rscratch", bufs=1, name="rsc")
            nc.vector.reciprocal_approx_accurate(
                out_ap, in_ap, sc[:, :out_ap.shape[-1]])

        def rstd_from_var(var_ps, d_dim):
            """psum var-sum [1,N] -> sbuf rstd [1,N] fp32."""
            r = misc.tile([1, var_ps.shape[-1]], F32, tag="rstd_tmp", bufs=2)
            nc.vector.tensor_scalar(
                r[:], var_ps[:], 1.0 / d_dim, EPS, OP.mult, OP.add
            )
            recip(r[:], r[:])
            nc.scalar.activation(r[:], r[:], AF.Sqrt)
            return r

        def bcast(row_ap, out_sb, wide_range=False, on_act=False):
            """[1,N] sbuf fp32 -> [P,N] sbuf fp32 via K=1 16-bit matmul.

            wide_range=True uses bf16 for the row (softmax denominators
            span e^0..e^17; 1/den underflows fp16 subnormals). The bf16
            cast runs on the idle gpsimd engine (the ACT queue is
            backlogged with exps mid-attention) unless on_act=True (used
            for the last heads, where gpsimd's ~1.9us latency would sit
            on the o-proj critical path and ACT is free again)."""
            n = row_ap.shape[-1]
            if wide_range:
                r16 = misc.tile([1, 512], BF16, tag="browb", bufs=2, name="rb")
                ones_row = ones_bfr
                if on_act:
                    nc.scalar.activation(r16[:, :n], row_ap, AF.Copy)
                else:
                    nc.gpsimd.tensor_copy(r16[:, :n], row_ap)
            else:
                r16 = misc.tile([1, 512], F16, tag="brow16", bufs=2, name="r16")
                ones_row = ones16r
                nc.scalar.activation(r16[:, :n], row_ap, AF.Copy)
            bc_ps = psum.tile([P, 512], F32, tag="big", bufs=7, name="bc_ps")
            nc.tensor.matmul(bc_ps[:, :n], ones_row[:], r16[:, :n],
                             start=True, stop=True)
            nc.vector.tensor_copy(out_sb, bc_ps[:, :n])

        # ================= phase 0: norms + K/V projections ================
        p_norm = tc.alloc_tile_pool(name="p_norm", bufs=1, side="left")
        p_qkv = tc.alloc_tile_pool(name="p_qkv", bufs=1, side="right")

        ident = misc.tile([P, P], F32, tag="ident")
        make_identity(nc, ident[:])
        hn = p_norm.tile([P, DC, TL], F16, tag="hn")
        hq = p_norm.tile([P, DC, TQ], F16, tag="hq")
        rdb1 = p_norm.tile([P, TL], F32, tag="rdb1")
        rdbq = p_norm.tile([P, TQ], F32, tag="rdbq")
        rstd_col = p_norm.tile([P, KCL], F32, tag="rstd_col")
        KT = p_qkv.tile([P, KVH, TKV], F16, tag="KT")
        V = p_qkv.tile([P, KC, DV], BF16, tag="V")
        with tc.tile_pool(name="s0", bufs=1, side="left") as s0:
            QT = p_qkv.tile([P, H, TQ], F16, tag="QT")

            def rope(ps, cos_ap, sin_ap, out_ap, n):
                """ps [128,n] psum fp32 (rows 0:64 = re, 64:128 = im,
                permuted), out_ap [128,n] fp16."""
                re, im = ps[0:64, :], ps[64:128, :]
                t1 = s0.tile([64, n], F32, tag="rope1", bufs=2)
                t2 = s0.tile([64, n], F32, tag="rope2", bufs=2)
                nc.vector.tensor_tensor(t1[:], re, cos_ap, OP.mult)
                nc.vector.tensor_tensor(t2[:], im, sin_ap, OP.mult)
                nc.vector.tensor_tensor(out_ap[0:64, :], t1[:], t2[:], OP.subtract)
                nc.vector.tensor_tensor(t1[:], re, sin_ap, OP.mult)
                nc.vector.tensor_tensor(t2[:], im, cos_ap, OP.mult)
                nc.vector.tensor_tensor(out_ap[64:128, :], t1[:], t2[:], OP.add)

            # ---- local kv-slice cast + rstd (kv side FIRST: everything up
            # to the AllGather trigger is on the critical path; the
            # query-side var/proj overlap the collective afterwards) ----
            vark_ps = psum.tile([1, 512], F32, tag="small", bufs=1, name="var_k")
            for dc in range(DC):
                nc.sync.dma_start(hn[:, dc, :],
                                  t["xT"][dc * P:(dc + 1) * P, :])
                sq = s0.tile([P, TL], F16, tag="sqk", bufs=3, name="sqk")
                nc.vector.tensor_tensor(sq[:], hn[:, dc, :], hn[:, dc, :],
                                        OP.mult)
                nc.tensor.matmul(vark_ps[:, :TL], ones16[:], sq[:],
                                 start=(dc == 0), stop=(dc == DC - 1))
            for dc in range(DC):
                nc.sync.dma_start(hq[:, dc, :],
                                  t["xq16"][dc * P:(dc + 1) * P, :])
            rk = rstd_from_var(vark_ps[:, :TL], D)
            bcast(rk[:], rdb1[:])
            # per-token rstd as a partition-indexed column (for V): transpose
            # of the broadcast tile is again a broadcast.
            for j in range(KCL):
                tp = psum.tile([P, 512], F32, tag="big", bufs=7, name="tp")
                nc.tensor.transpose(tp[:, :P],
                                    rdb1[:, j * P:(j + 1) * P], ident[:])
                nc.vector.tensor_copy(rstd_col[:, j:j + 1], tp[:, 0:1])

            cosk = s0.tile([64, TL], F32, tag="cosk")
            nc.sync.dma_start(cosk[:], t["cos_k"][:])
            sink = s0.tile([64, TL], F32, tag="sink")
            nc.sync.dma_start(sink[:], t["sin_k"][:])
            nc.vector.tensor_tensor(cosk[:], cosk[:], rdb1[:64, :], OP.mult)
            nc.vector.tensor_tensor(sink[:], sink[:], rdb1[:64, :], OP.mult)

            # ---- K projection for the local slice ----
            KT_loc = s0.tile([P, KVH, TL], F16, tag="KT_loc")
            for et in range(KVH):
                wk_sb = s0.tile([P, DC, P], F16, tag="wkq", bufs=4)
                nc.scalar.dma_start(wk_sb[:], t["wk_t"][et])
                pk = psum.tile([P, 512], F32, tag="big", bufs=7, name="pk")
                for dc in range(DC):
                    nc.tensor.matmul(
                        pk[:, :TL], wk_sb[:, dc, :], hn[:, dc, :],
                        start=(dc == 0), stop=(dc == DC - 1),
                    )
                rope(pk[:, :TL], cosk[:], sink[:], KT_loc[:, et, :], TL)

            # ---- V projection for the local slice ----
            V_loc = s0.tile([P, KCL, DV], BF16, tag="V_loc")
            wv_sb = s0.tile([P, DC, DV], F16, tag="wv")
            for dc in range(DC):
                nc.scalar.dma_start(wv_sb[:, dc, :], t["wv_r"][dc])
            for tt in range(KCL):
                pv = psum.tile([P, 512], F32, tag="big", bufs=7, name="pv")
                for dc in range(DC):
                    nc.tensor.matmul(
                        pv[:, :DV], hn[:, dc, tt * P:(tt + 1) * P],
                        wv_sb[:, dc, :],
                        start=(dc == 0), stop=(dc == DC - 1),
                    )
                nc.scalar.activation(V_loc[:, tt, :], pv[:, :DV], AF.Copy,
                                     scale=rstd_col[:, tt:tt + 1])

            # ---- AllGather K/V across the 4-core group (HBM bounce) ----
            nc.gpsimd.dma_start(t["kvb_in"][:, :KVH, :], KT_loc[:])
            nc.gpsimd.dma_start(t["kvb_in"][:, KVH:, :], V_loc[:].bitcast(F16))
            nc.gpsimd.collective_compute(
                "AllGather", OP.bypass, replica_groups=REPLICA_GROUPS,
                ins=[t["kvb_in"][:]], outs=[t["kvb_out"][:]],
            )

            # ---- query-token cast + rstd (overlaps the gather) ----
            varq_ps = psum.tile([1, 512], F32, tag="small", bufs=1, name="var_q")
            for dc in range(DC):
                sq = s0.tile([P, TQ], F16, tag="sqq", bufs=3, name="sqq")
                nc.vector.tensor_tensor(sq[:], hq[:, dc, :], hq[:, dc, :],
                                        OP.mult)
                nc.tensor.matmul(varq_ps[:, :TQ], ones16[:], sq[:],
                                 start=(dc == 0), stop=(dc == DC - 1))
            rq = rstd_from_var(varq_ps[:, :TQ], D)
            bcast(rq[:], rdbq[:])
            cosq = s0.tile([64, TQ], F32, tag="cosq")
            nc.sync.dma_start(cosq[:], t["cos_q"][:])
            sinq = s0.tile([64, TQ], F32, tag="sinq")
            nc.sync.dma_start(sinq[:], t["sin_q"][:])
            nc.vector.tensor_tensor(cosq[:], cosq[:], rdbq[:64, :], OP.mult)
            nc.vector.tensor_tensor(sinq[:], sinq[:], rdbq[:64, :], OP.mult)

            # ---- phase 1: Q projections (overlap the gather) ----
            for et in range(H):
                wq_sb = s0.tile([P, DC, P], F16, tag="wkq", bufs=4)
                nc.scalar.dma_start(wq_sb[:], t["wq_t"][et])
                pq = psum.tile([P, 512], F32, tag="big", bufs=7, name="pq")
                for dc in range(DC):
                    nc.tensor.matmul(
                        pq[:, :TQ], wq_sb[:, dc, :], hq[:, dc, :],
                        start=(dc == 0), stop=(dc == DC - 1),
                    )
                rope(pq[:, :TQ], cosq[:], sinq[:], QT[:, et, :], TQ)

            # gathered K/V readback: emitted after the Q weight DMAs so these
            # sem-gated transfers don't head-of-line-block the weight queues;
            # K slots first (scores need K before AV needs V).
            for r in range(4):
                for et in range(KVH):
                    nc.scalar.dma_start(KT[:, et, r * TL:(r + 1) * TL],
                                        t["kvb_out"][r, :, et, :])
            for r in range(4):
                for ch in range(KCL):
                    nc.scalar.dma_start(V[:, KCL * r + ch, :].bitcast(F16),
                                        t["kvb_out"][r, :, KVH + ch, :])

        p_norm.release()  # hn/hq dead

        # ================= phase 2: attention ==============================
        n_rep = H // KVH
        with tc.tile_pool(name="p_att", bufs=1, side="left") as p_att, \
                tc.tile_pool(name="s3", bufs=1, side="left") as s3:
            # in ascending-extent block order only the first 64-query block
            # of each chunk's active range can be masked/invalid; the rest
            # is always fully visible.
            mask = p_att.tile([P, KC, BLK], BF16, tag="mask")
            for kc in range(KC):
                nc.sync.dma_start(mask[:, kc, :], t["maskb"][kc])
            attnT = p_att.tile([P, H, TQ], F16, tag="attnT")
            # two-deep software pipeline across heads: scores/exp/mask of
            # head h run while AV+denominator matmuls of head h-1 and the
            # normalize of head h-2 retire. The in-order PE then never waits
            # on the ACT exp chain or the DVE reciprocal.
            def emit_scores(h):
                g = h // n_rep
                es = p_att.tile([P, KC, TQ], BF16, tag="expS", bufs=3, name="es")
                # narrow chunks first: the wide chunks' slower exp evictions
                # then drain while the next pipeline stage's AV matmuls run.
                # Chunk pairs (2m, 2m+1) share offset/width; for kc >= 8 both
                # fit in one psum bank and take a single (cheaper) exp.
                for kc0 in (14, 12, 10, 8):
                    o = OFF[kc0]
                    w = TQ - o
                    ps = psum.tile([P, 2, 256], F32, tag="big", bufs=7,
                                   name="ps")
                    for i in (0, 1):
                        nc.tensor.matmul(
                            ps[:, i, 0:w],
                            KT[:, g, (kc0 + i) * P:(kc0 + i + 1) * P],
                            QT[:, h, o:TQ],
                            start=True, stop=True, skip_group_check=True,
                        )
                    nc.scalar.activation(es[:, kc0:kc0 + 2, o:],
                                         ps[:, :, 0:w], AF.Exp)
                    for i in (0, 1):
                        nc.vector.tensor_tensor(
                            es[:, kc0 + i, o:o + BLK],
                            es[:, kc0 + i, o:o + BLK],
                            mask[:, kc0 + i, :], OP.mult)
                for kc in reversed(range(8)):
                    o = OFF[kc]
                    ps = psum.tile([P, 512], F32, tag="big", bufs=7, name="ps")
                    nc.tensor.matmul(
                        ps[:, o:TQ], KT[:, g, kc * P:(kc + 1) * P],
                        QT[:, h, o:TQ],
                        start=True, stop=True,
                    )
                    nc.scalar.activation(es[:, kc, o:], ps[:, o:TQ], AF.Exp)
                    nc.vector.tensor_tensor(es[:, kc, o:o + BLK],
                                            es[:, kc, o:o + BLK],
                                            mask[:, kc, :], OP.mult)
                return es

            def emit_av_den(h, es):
                g = h // n_rep
                pav = psum.tile([P, 512], F32, tag="big", bufs=7, name="pav")
                pden = psum.tile([1, 512], F32, tag="small", bufs=1, name="pden")
                for kc in range(KC):
                    o = OFF[kc]
                    nc.tensor.matmul(pden[:, o:TQ], ones_bf[:], es[:, kc, o:],
                                     start=(kc == 0), stop=(kc == KC - 1),
                                     skip_group_check=True)
                for kc in range(KC):
                    o = OFF[kc]
                    nc.tensor.matmul(
                        pav[:, o:TQ], V[:, kc, g * P:(g + 1) * P],
                        es[:, kc, o:],
                        start=(kc == 0), stop=(kc == KC - 1),
                        skip_group_check=True,
                    )
                rden = misc.tile([1, TQ], F32, tag="rstd_tmp", bufs=2, name="rden")
                recip(rden[:], pden[:, :TQ])
                return pav, rden

            def finish_head(ph, ppav, prden, on_act=False):
                # gpsimd ucode broadcast: replaces the fp16-cast + K=1 PE
                # matmul + DVE psum copy, keeps rden fp32, and frees a psum
                # bank per head. Latency (~us) hides in the 2-deep pipeline.
                rdba = p_att.tile([P, TQ], F32, tag="rdba", bufs=2)
                nc.gpsimd.partition_broadcast(rdba[:], prden[:, :TQ])
                nc.vector.tensor_tensor(attnT[:, ph, :], ppav[:, :TQ], rdba[:],
                                        OP.mult)

            fin1 = None  # (h, pav, rden) awaiting bcast+normalize
            fin0 = None  # (h, es) awaiting av+den
            for h in range(H):
                es = emit_scores(h)
                if fin1 is not None:
                    finish_head(*fin1)
                    fin1 = None
                if fin0 is not None:
                    ph, pes = fin0
                    ppav, prden = emit_av_den(ph, pes)
                    fin1 = (ph, ppav, prden)
                fin0 = (h, es)
            ph, pes = fin0
            if fin1 is not None:
                finish_head(*fin1)
            ppav, prden = emit_av_den(ph, pes)
            finish_head(ph, ppav, prden, on_act=True)

            p_qkv.release()  # KT/QT/V dead

            # ============= phase 3: o-proj + residual + RMSNorm2 ===========
            p_res = ctx.enter_context(
                tc.tile_pool(name="p_res", bufs=1, side="right"))
            h2 = p_res.tile([P, DC, TQ], F32, tag="h2")
            mt = p_res.tile([P, DC, TQ], F16, tag="mt")
            var2_ps = psum.tile([1, 512], F32, tag="small", bufs=1, name="var2")
            for dt in range(DC):
                wo_sb = s3.tile([P, H, P], F16, tag="wo", bufs=3)
                nc.sync.dma_start(wo_sb[:], t["wo_t"][dt])
                po = psum.tile([P, 512], F32, tag="big", bufs=7, name="po")
                for ec in range(H):
                    nc.tensor.matmul(
                        po[:, :TQ], wo_sb[:, ec, :], attnT[:, ec, :],
                        start=(ec == 0), stop=(ec == H - 1),
                    )
                xqr = s3.tile([P, TQ], F32, tag="xq2", bufs=2)
                nc.sync.dma_start(xqr[:], t["xq"][dt * P:(dt + 1) * P, :])
                nc.vector.tensor_tensor(h2[:, dt, :], po[:, :TQ], xqr[:], OP.add)
                nc.scalar.activation(mt[:, dt, :], h2[:, dt, :], AF.Copy)
                sq = s3.tile([P, TQ], F16, tag="sq3", bufs=3)
                nc.vector.tensor_tensor(sq[:], h2[:, dt, :], h2[:, dt, :],
                                        OP.mult)
                nc.tensor.matmul(var2_ps[:, :TQ], ones16[:], sq[:],
                                 start=(dt == 0), stop=(dt == DC - 1))
            r2 = rstd_from_var(var2_ps[:, :TQ], D)
            rdb2 = p_res.tile([P, TQ], F32, tag="rdb2")
            bcast(r2[:], rdb2[:])

        # ================= phase 4: MLP gate/up + silu =====================
        with tc.tile_pool(name="p_gu", bufs=1, side="left") as p_gu:
            gu = p_gu.tile([P, IT, TQ], F16, tag="gu")
            with tc.tile_pool(name="s45", bufs=1, side="left") as s4:
                s5 = s4
                for it in range(IT):
                    wg_sb = s4.tile([P, DC, P], F16, tag="wgu", bufs=4)
                    nc.sync.dma_start(wg_sb[:], t["wg_t"][it])
                    wu_sb = s4.tile([P, DC, P], F16, tag="wgu", bufs=4)
                    nc.sync.dma_start(wu_sb[:], t["wu_t"][it])
                    pg = psum.tile([P, 512], F32, tag="big", bufs=7, name="pg")
                    for dc in range(DC):
                        nc.tensor.matmul(pg[:, :TQ], wg_sb[:, dc, :], mt[:, dc, :],
                                         start=(dc == 0), stop=(dc == DC - 1))
                    pu = psum.tile([P, 512], F32, tag="big", bufs=7, name="pu")
                    for dc in range(DC):
                        nc.tensor.matmul(pu[:, :TQ], wu_sb[:, dc, :], mt[:, dc, :],
                                         start=(dc == 0), stop=(dc == DC - 1))
                    # raw gate/up are projections of the unnormalized h2;
                    # apply rstd2 at the nonlinearity (t1 = g*r):
                    # silu(g*r)*(u*r) = sig(t1)*t1*u*r.
                    t1 = s4.tile([P, TQ], F32, tag="t1", bufs=3)
                    nc.vector.tensor_tensor(t1[:], pg[:, :TQ], rdb2[:], OP.mult)
                    sg = s4.tile([P, TQ], F16, tag="sg", bufs=3)
                    nc.scalar.activation(sg[:], t1[:], AF.Sigmoid)
                    t2 = s4.tile([P, TQ], F16, tag="gg", bufs=3)
                    nc.vector.tensor_tensor(t2[:], sg[:], pu[:, :TQ], OP.mult)
                    t3 = s4.tile([P, TQ], F16, tag="t3", bufs=3)
                    nc.vector.tensor_tensor(t3[:], t1[:], t2[:], OP.mult)
                    nc.vector.tensor_tensor(gu[:, it, :], t3[:], rdb2[:], OP.mult)

                # ============= phase 5: MLP down + residual ================
                for dt in range(DC):
                    wd_sb = s5.tile([P, IT, P], F16, tag="wd", bufs=2)
                    nc.sync.dma_start(wd_sb[:], t["wd_t"][dt])
                    pd = psum.tile([P, 512], F32, tag="big", bufs=7, name="pd")
                    for ic in range(IT):
                        nc.tensor.matmul(pd[:, :TQ], wd_sb[:, ic, :], gu[:, ic, :],
                                         start=(ic == 0), stop=(ic == IT - 1))
                    outp = s5.tile([P, TQ], F32, tag="out", bufs=3)
                    nc.vector.tensor_tensor(outp[:], pd[:, :TQ], h2[:, dt, :],
                                            OP.add)
                    nc.sync.dma_start(t["outT"][dt * P:(dt + 1) * P, :], outp[:])


# --------------------------------------------------------------------------
# host-side input prep
# --------------------------------------------------------------------------

def _permute_heads(w, nheads):
    """Reorder each head's 128 rows as [even dims, odd dims] so RoPE's
    interleaved pairs become contiguous halves on-chip."""
    perm = np.concatenate([np.arange(0, P, 2), np.arange(1, P, 2)])
    return w.reshape(nheads, P, -1)[:, perm, :].reshape(nheads * P, -1)


def prep_weights(cfg, wq, wk, wv, wo, w_gate, w_up, w_down, ln1_w, ln2_w):
    D, H, KVH, I = cfg["D"], cfg["H"], cfg["KVH"], cfg["I"]
    DC, IT = D // P, I // P
    f16 = np.float16
    c = np.ascontiguousarray

    wq_p = _permute_heads(wq * ln1_w[None, :], H)
    wk_p = _permute_heads(wk * ln1_w[None, :], KVH)
    wv_f = wv * ln1_w[None, :]
    wg_f = w_gate * ln2_w[None, :]
    wu_f = w_up * ln2_w[None, :]

    out = {}
    # lhsT tile layouts: [outer_tile, partition(128), inner_seq, free(128)]
    out["wq_t"] = c(wq_p.reshape(H, P, DC, P).transpose(0, 3, 2, 1).astype(f16))
    out["wk_t"] = c(wk_p.reshape(KVH, P, DC, P).transpose(0, 3, 2, 1).astype(f16))
    out["wv_r"] = c(wv_f.T.reshape(DC, P, KVH * P).astype(f16))
    out["wo_t"] = c(wo.reshape(DC, P, H, P).transpose(0, 3, 2, 1).astype(f16))
    out["wg_t"] = c(wg_f.reshape(IT, P, DC, P).transpose(0, 3, 2, 1).astype(f16))
    out["wu_t"] = c(wu_f.reshape(IT, P, DC, P).transpose(0, 3, 2, 1).astype(f16))
    out["wd_t"] = c(w_down.reshape(DC, P, IT, P).transpose(0, 3, 2, 1).astype(f16))
    return out


def core_query_index(core):
    """Balanced causal query blocks for a core, ascending-extent order."""
    j = core % 4
    blocks = [j, 7 - j, 8 + j, 15 - j, 16 + j, 23 - j, 24 + j, 31 - j]
    return np.concatenate([np.arange(BLK * s, BLK * (s + 1)) for s in blocks])


def prep_core_inputs(cfg, core, weights, hidden_states, cos, sin, attention_mask):
    """Per-core activation slices. core -> (batch, paired query sub-blocks)."""
    TQ, TKV = cfg["TQ"], cfg["TKV"]
    n_chunk = TKV // TQ
    b = core // n_chunk
    q = core % n_chunk
    ks = slice(q * TQ, (q + 1) * TQ)       # contiguous kv slice for this core
    qidx = core_query_index(core)
    scale = 128.0 ** -0.5
    c = np.ascontiguousarray
    f32 = np.float32

    m = dict(weights)
    xT = c(hidden_states[b].T.astype(f32))
    m["xT"] = c(xT[:, ks].astype(np.float16))
    m["xq"] = c(xT[:, qidx])
    m["xq16"] = c(xT[:, qidx].astype(np.float16))
    m["cos_k"] = c(cos[b, ks, :64].T.astype(f32))
    m["sin_k"] = c(sin[b, ks, :64].T.astype(f32))
    m["cos_q"] = c(cos[b, qidx, :64].T.astype(f32) * scale)
    m["sin_q"] = c(sin[b, qidx, :64].T.astype(f32) * scale)
    # per-chunk causal mask for the one 64-query block (index kc//2 in the
    # ascending-extent order) that can be partial/invalid at chunk kc; all
    # other active blocks are fully visible by construction.
    am = attention_mask[b, 0]
    kc_n = TKV // P
    mask = np.zeros((kc_n, P, BLK), ml_dtypes.bfloat16)
    for kc in range(kc_n):
        qi = qidx[BLK * (kc // 2):BLK * (kc // 2 + 1)]
        mask[kc] = (am[np.ix_(qi, np.arange(kc * P, (kc + 1) * P))]
                    > NEG_THRESH).astype(ml_dtypes.bfloat16).T
    m["maskb"] = mask
    return m


# --------------------------------------------------------------------------
# entry point
# --------------------------------------------------------------------------

_NC_CACHE = {}


def _get_nc(cfg_key):
    if cfg_key not in _NC_CACHE:
        _NC_CACHE[cfg_key] = build_nc(FULL_CFG)
    return _NC_CACHE[cfg_key]


def kernel(hidden_states, cos, sin, attention_mask,
           wq, wk, wv, wo, w_gate, w_up, w_down, ln1_w, ln2_w):
    global LAST_EXEC_NS
    cfg = FULL_CFG
    nc = _get_nc("full")

    weights = prep_weights(
        cfg,
        np.asarray(wq, np.float32), np.asarray(wk, np.float32),
        np.asarray(wv, np.float32), np.asarray(wo, np.float32),
        np.asarray(w_gate, np.float32), np.asarray(w_up, np.float32),
        np.asarray(w_down, np.float32),
        np.asarray(ln1_w, np.float32), np.asarray(ln2_w, np.float32),
    )
    hs = np.asarray(hidden_states, np.float32)
    cos = np.asarray(cos, np.float32)
    sin = np.asarray(sin, np.float32)
    am = np.asarray(attention_mask, np.float32)

    in_maps = [prep_core_inputs(cfg, c, weights, hs, cos, sin, am)
               for c in range(8)]

    trace = bool(int(os.environ.get("KERNEL_TRACE", "0")))
    trace_cores = None
    if trace and os.environ.get("KERNEL_TRACE_ALL"):
        trace_cores = list(range(8))
    res = run_bass_kernel_spmd(
        nc, in_maps, core_ids=list(range(8)), trace=trace,
        trace_cores=trace_cores,
        tmpdir=os.environ.get("KERNEL_TRACE_DIR") or None,
    )
    LAST_EXEC_NS = res.exec_time_ns

    B, S = hs.shape[0], hs.shape[1]
    n_chunk = cfg["TKV"] // cfg["TQ"]
    out = np.empty((B, S, cfg["D"]), np.float32)
    for c in range(8):
        b = c // n_chunk
        out[b, core_query_index(c), :] = res.results[c]["outT"].T
    return out


# revision 30
# speedup vs baseline: 1.0079x; 1.0079x over previous
"""Trainium2 Bass kernel for one BLT transformer layer (B=2, S=2048, D=2048,
H=16, KVH=4, HD=128, I=8192, fp32 I/O).

Sharding: sequence-parallel over 8 cores with one K/V AllGather. Core c
handles batch b=c//4 and 512 query tokens chosen as 8 causally-balanced
64-token blocks. K/V projection work is deduplicated: each core computes
K/V (+RMSNorm rstd) only for its contiguous 512-token kv slice, then the
4 cores of a batch AllGather the rope'd/normalized K (fp16) and V (bf16)
through HBM bounce buffers while the Q projections keep the PE busy.
Attention (causal-skip), o-proj, and the full MLP for the core's 512
query tokens are unchanged from the no-collective version.

On-chip layout is feature-major [feature, token] throughout, so every
matmul contracts along the partition dim with no on-chip transposes.
Matmuls run in fp16 (fp32 PSUM accumulation); softmax/norm math in fp32.
The rstd broadcasts ride a K=1 fp16 matmul (fp32 rows cost 4 cycles/col
on the PE); the per-head softmax normalizer is broadcast by a gpsimd
ucode op instead, keeping the wide-range 1/den in fp32 (it spans e^-17..
e^0 and underflows fp16) and freeing a psum bank per head. RoPE's interleaved pairs become contiguous halves via a
host-side even/odd permutation of the wq/wk rows. Softmax skips the
max-subtraction (exp fits in bf16) and applies the causal mask as a
binary multiply on exp(scores); with the ascending-extent block order
only one 64-query block per kv chunk can be partial/invalid, so the mask
multiply touches just 64 columns per chunk.
"""

import os
from contextlib import ExitStack

import ml_dtypes
import numpy as np

import concourse.bacc as bacc
import concourse.mybir as mybir
import concourse.tile as tile
from concourse.bass_utils import run_bass_kernel_spmd
from concourse.masks import make_identity

F8 = mybir.dt.float8e4
F16 = mybir.dt.float16
BF16 = mybir.dt.bfloat16
F32 = mybir.dt.float32
AF = mybir.ActivationFunctionType
OP = mybir.AluOpType
DR = mybir.MatmulPerfMode.DoubleRow

P = 128
EPS = 1e-6
NEG_THRESH = -0.5  # additive mask values are 0.0 or -1e9

FULL_CFG = dict(D=2048, TKV=2048, TQ=512, H=16, KVH=4, I=8192)

# Causal-skip attention. Each core handles 8 query blocks of 64 tokens from
# one batch, chosen so the multiset of causal extents is balanced across
# cores (blocks {j, 7-j, 8+j, 15-j, 16+j, 23-j, 24+j, 31-j} for j = core%4)
# and laid out in the 512-query space in ascending-extent order. A single
# SPMD program must have one shape, so the static per-chunk active width is
# the max extent over cores: kv chunk kc only needs query columns
# [64*(kc//2), 512). Scores/AV/denominator instruction count stays at the
# dense kernel's 16 per head, but columns drop to 56.25%; per-core validity
# inside the active width is data-driven via the host-built causal mask.
BLK = 64
N_BLK = 32                               # 64-token blocks per batch
OFF = [BLK * (kc // 2) for kc in range(16)]   # first active column per chunk

REPLICA_GROUPS = [[0, 1, 2, 3], [4, 5, 6, 7]]

LAST_EXEC_NS = None


# --------------------------------------------------------------------------
# kernel body (built once per process)
# --------------------------------------------------------------------------

def build_nc(cfg, debug=False):
    D, TKV, TQ, H, KVH, I = (cfg[k] for k in ("D", "TKV", "TQ", "H", "KVH", "I"))
    DC = D // P          # d-model chunks
    KC = TKV // P        # kv-token chunks (full batch)
    IT = I // P          # intermediate tiles
    DV = KVH * P         # v width
    TL = TQ              # local kv-slice length (512 tokens per core)
    KCL = TL // P        # local kv chunks
    assert TQ <= 512 and DV <= 512

    nc = bacc.Bacc("TRN2", target_bir_lowering=False, debug=debug)

    t = {}
    t["xT"] = nc.dram_tensor("xT", [D, TL], F16, kind="ExternalInput")
    t["xq"] = nc.dram_tensor("xq", [D, TQ], F32, kind="ExternalInput")
    t["xq16"] = nc.dram_tensor("xq16", [D, TQ], F16, kind="ExternalInput")
    t["cos_q"] = nc.dram_tensor("cos_q", [64, TQ], F32, kind="ExternalInput")
    t["sin_q"] = nc.dram_tensor("sin_q", [64, TQ], F32, kind="ExternalInput")
    t["cos_k"] = nc.dram_tensor("cos_k", [64, TL], F32, kind="ExternalInput")
    t["sin_k"] = nc.dram_tensor("sin_k", [64, TL], F32, kind="ExternalInput")
    t["maskb"] = nc.dram_tensor("maskb", [TKV // P, P, BLK], BF16,
                                kind="ExternalInput")
    t["wq_t"] = nc.dram_tensor("wq_t", [H, P, DC, P], F16, kind="ExternalInput")
    t["wk_t"] = nc.dram_tensor("wk_t", [KVH, P, DC, P], F16, kind="ExternalInput")
    t["wv_r"] = nc.dram_tensor("wv_r", [DC, P, DV], F16, kind="ExternalInput")
    t["wo_t"] = nc.dram_tensor("wo_t", [DC, P, H, P], F16, kind="ExternalInput")
    t["wg_t"] = nc.dram_tensor("wg_t", [IT, P, DC, P], F16, kind="ExternalInput")
    t["wu_t"] = nc.dram_tensor("wu_t", [IT, P, DC, P], F16, kind="ExternalInput")
    t["wd_t"] = nc.dram_tensor("wd_t", [DC, P, IT, P], F16, kind="ExternalInput")
    t["outT"] = nc.dram_tensor("outT", [D, TQ], F32, kind="ExternalOutput")
    # HBM bounce buffers for the K/V AllGather within each 4-core group.
    # One combined collective: slots 0..KVH-1 hold K (fp16), slots
    # KVH..KVH+KCL-1 hold V (bf16 bitcast to fp16 bytes).
    t["kvb_in"] = nc.dram_tensor("kvb_in", [P, KVH + KCL, TL], F16,
                                 kind="Internal")
    t["kvb_out"] = nc.dram_tensor("kvb_out", [4, P, KVH + KCL, TL], F16,
                                  kind="Internal")

    with tile.TileContext(nc) as tc:
        _body(nc, tc, t, D, TKV, TQ, H, KVH, I, DC, KC, IT, DV, TL, KCL)
    nc.compile()
    return nc


def _body(nc, tc, t, D, TKV, TQ, H, KVH, I, DC, KC, IT, DV, TL, KCL):
    with ExitStack() as ctx:
        # global pools: small constants + one PSUM pool budgeted to 8 banks
        # (proj 3 + scores 2 + av 2 + small 1).
        misc = ctx.enter_context(tc.tile_pool(name="misc", bufs=1, side="right"))
        psum = ctx.enter_context(tc.tile_pool(name="psum", bufs=1, space="PSUM"))

        ones16 = misc.tile([P, 1], F16, tag="ones16")
        nc.vector.memset(ones16[:], 1.0)
        ones16r = misc.tile([1, P], F16, tag="ones16r")
        nc.vector.memset(ones16r[:], 1.0)
        ones_bfr = misc.tile([1, P], BF16, tag="ones_bfr")
        nc.vector.memset(ones_bfr[:], 1.0)
        ones_bf = misc.tile([P, 1], BF16, tag="ones_bf")
        nc.vector.memset(ones_bf[:], 1.0)

        def recip(out_ap, in_ap):
            sc = misc.tile([1, 512], F32, tag="rscratch", bufs=1, name="rsc")
            nc.vector.reciprocal_approx_accurate(
                out_ap, in_ap, sc[:, :out_ap.shape[-1]])

        def rstd_from_var(var_ps, d_dim):
            """psum var-sum [1,N] -> sbuf rstd [1,N] fp32."""
            r = misc.tile([1, var_ps.shape[-1]], F32, tag="rstd_tmp", bufs=2)
            nc.vector.tensor_scalar(
                r[:], var_ps[:], 1.0 / d_dim, EPS, OP.mult, OP.add
            )
            recip(r[:], r[:])
            nc.scalar.activation(r[:], r[:], AF.Sqrt)
            return r

        def bcast(row_ap, out_sb, wide_range=False, on_act=False):
            """[1,N] sbuf fp32 -> [P,N] sbuf fp32 via K=1 16-bit matmul.

            wide_range=True uses bf16 for the row (softmax denominators
            span e^0..e^17; 1/den underflows fp16 subnormals). The bf16
            cast runs on the idle gpsimd engine (the ACT queue is
            backlogged with exps mid-attention) unless on_act=True (used
            for the last heads, where gpsimd's ~1.9us latency would sit
            on the o-proj critical path and ACT is free again)."""
            n = row_ap.shape[-1]
            if wide_range:
                r16 = misc.tile([1, 512], BF16, tag="browb", bufs=2, name="rb")
                ones_row = ones_bfr
                if on_act:
                    nc.scalar.activation(r16[:, :n], row_ap, AF.Copy)
                else:
                    nc.gpsimd.tensor_copy(r16[:, :n], row_ap)
            else:
                r16 = misc.tile([1, 512], F16, tag="brow16", bufs=2, name="r16")
                ones_row = ones16r
                nc.scalar.activation(r16[:, :n], row_ap, AF.Copy)
            bc_ps = psum.tile([P, 512], F32, tag="big", bufs=7, name="bc_ps")
            nc.tensor.matmul(bc_ps[:, :n], ones_row[:], r16[:, :n],
                             start=True, stop=True)
            nc.vector.tensor_copy(out_sb, bc_ps[:, :n])

        # ================= phase 0: norms + K/V projections ================
        p_norm = tc.alloc_tile_pool(name="p_norm", bufs=1, side="left")
        p_qkv = tc.alloc_tile_pool(name="p_qkv", bufs=1, side="right")

        ident = misc.tile([P, P], F32, tag="ident")
        make_identity(nc, ident[:])
        hn = p_norm.tile([P, DC, TL], F16, tag="hn")
        hq = p_norm.tile([P, DC, TQ], F16, tag="hq")
        rdb1 = p_norm.tile([P, TL], F32, tag="rdb1")
        rdbq = p_norm.tile([P, TQ], F32, tag="rdbq")
        rstd_col = p_norm.tile([P, KCL], F32, tag="rstd_col")
        KT = p_qkv.tile([P, KVH, TKV], F16, tag="KT")
        V = p_qkv.tile([P, KC, DV], BF16, tag="V")
        with tc.tile_pool(name="s0", bufs=1, side="left") as s0:
            QT = p_qkv.tile([P, H, TQ], F16, tag="QT")

            def rope(ps, cos_ap, sin_ap, out_ap, n):
                """ps [128,n] psum fp32 (rows 0:64 = re, 64:128 = im,
                permuted), out_ap [128,n] fp16."""
                re, im = ps[0:64, :], ps[64:128, :]
                t1 = s0.tile([64, n], F32, tag="rope1", bufs=2)
                t2 = s0.tile([64, n], F32, tag="rope2", bufs=2)
                nc.vector.tensor_tensor(t1[:], re, cos_ap, OP.mult)
                nc.vector.tensor_tensor(t2[:], im, sin_ap, OP.mult)
                nc.vector.tensor_tensor(out_ap[0:64, :], t1[:], t2[:], OP.subtract)
                nc.vector.tensor_tensor(t1[:], re, sin_ap, OP.mult)
                nc.vector.tensor_tensor(t2[:], im, cos_ap, OP.mult)
                nc.vector.tensor_tensor(out_ap[64:128, :], t1[:], t2[:], OP.add)

            # ---- local kv-slice cast + rstd (kv side FIRST: everything up
            # to the AllGather trigger is on the critical path; the
            # query-side var/proj overlap the collective afterwards) ----
            vark_ps = psum.tile([1, 512], F32, tag="small", bufs=1, name="var_k")
            for dc in range(DC):
                nc.sync.dma_start(hn[:, dc, :],
                                  t["xT"][dc * P:(dc + 1) * P, :])
                sq = s0.tile([P, TL], F16, tag="sqk", bufs=3, name="sqk")
                nc.vector.tensor_tensor(sq[:], hn[:, dc, :], hn[:, dc, :],
                                        OP.mult)
                nc.tensor.matmul(vark_ps[:, :TL], ones16[:], sq[:],
                                 start=(dc == 0), stop=(dc == DC - 1))
            for dc in range(DC):
                nc.sync.dma_start(hq[:, dc, :],
                                  t["xq16"][dc * P:(dc + 1) * P, :])
            rk = rstd_from_var(vark_ps[:, :TL], D)
            bcast(rk[:], rdb1[:])
            # per-token rstd as a partition-indexed column (for V): transpose
            # of the broadcast tile is again a broadcast.
            for j in range(KCL):
                tp = psum.tile([P, 512], F32, tag="big", bufs=7, name="tp")
                nc.tensor.transpose(tp[:, :P],
                                    rdb1[:, j * P:(j + 1) * P], ident[:])
                nc.vector.tensor_copy(rstd_col[:, j:j + 1], tp[:, 0:1])

            cosk = s0.tile([64, TL], F32, tag="cosk")
            nc.sync.dma_start(cosk[:], t["cos_k"][:])
            sink = s0.tile([64, TL], F32, tag="sink")
            nc.sync.dma_start(sink[:], t["sin_k"][:])
            nc.vector.tensor_tensor(cosk[:], cosk[:], rdb1[:64, :], OP.mult)
            nc.vector.tensor_tensor(sink[:], sink[:], rdb1[:64, :], OP.mult)

            # ---- K projection for the local slice ----
            KT_loc = s0.tile([P, KVH, TL], F16, tag="KT_loc")
            for et in range(KVH):
                wk_sb = s0.tile([P, DC, P], F16, tag="wkq", bufs=4)
                nc.sync.dma_start(wk_sb[:], t["wk_t"][et])
                pk = psum.tile([P, 512], F32, tag="big", bufs=7, name="pk")
                for dc in range(DC):
                    nc.tensor.matmul(
                        pk[:, :TL], wk_sb[:, dc, :], hn[:, dc, :],
                        start=(dc == 0), stop=(dc == DC - 1),
                    )
                rope(pk[:, :TL], cosk[:], sink[:], KT_loc[:, et, :], TL)

            # ---- V projection for the local slice ----
            V_loc = s0.tile([P, KCL, DV], BF16, tag="V_loc")
            wv_sb = s0.tile([P, DC, DV], F16, tag="wv")
            for dc in range(DC):
                nc.sync.dma_start(wv_sb[:, dc, :], t["wv_r"][dc])
            for tt in range(KCL):
                pv = psum.tile([P, 512], F32, tag="big", bufs=7, name="pv")
                for dc in range(DC):
                    nc.tensor.matmul(
                        pv[:, :DV], hn[:, dc, tt * P:(tt + 1) * P],
                        wv_sb[:, dc, :],
                        start=(dc == 0), stop=(dc == DC - 1),
                    )
                nc.scalar.activation(V_loc[:, tt, :], pv[:, :DV], AF.Copy,
                                     scale=rstd_col[:, tt:tt + 1])

            # ---- AllGather K/V across the 4-core group (HBM bounce) ----
            nc.sync.dma_start(t["kvb_in"][:, :KVH, :], KT_loc[:])
            nc.sync.dma_start(t["kvb_in"][:, KVH:, :], V_loc[:].bitcast(F16))
            nc.gpsimd.collective_compute(
                "AllGather", OP.bypass, replica_groups=REPLICA_GROUPS,
                ins=[t["kvb_in"][:]], outs=[t["kvb_out"][:]],
            )

            # ---- query-token cast + rstd (overlaps the gather) ----
            varq_ps = psum.tile([1, 512], F32, tag="small", bufs=1, name="var_q")
            for dc in range(DC):
                sq = s0.tile([P, TQ], F16, tag="sqq", bufs=3, name="sqq")
                nc.vector.tensor_tensor(sq[:], hq[:, dc, :], hq[:, dc, :],
                                        OP.mult)
                nc.tensor.matmul(varq_ps[:, :TQ], ones16[:], sq[:],
                                 start=(dc == 0), stop=(dc == DC - 1))
            rq = rstd_from_var(varq_ps[:, :TQ], D)
            bcast(rq[:], rdbq[:])
            cosq = s0.tile([64, TQ], F32, tag="cosq")
            nc.sync.dma_start(cosq[:], t["cos_q"][:])
            sinq = s0.tile([64, TQ], F32, tag="sinq")
            nc.sync.dma_start(sinq[:], t["sin_q"][:])
            nc.vector.tensor_tensor(cosq[:], cosq[:], rdbq[:64, :], OP.mult)
            nc.vector.tensor_tensor(sinq[:], sinq[:], rdbq[:64, :], OP.mult)

            # ---- phase 1: Q projections (overlap the gather) ----
            for et in range(H):
                wq_sb = s0.tile([P, DC, P], F16, tag="wkq", bufs=4)
                nc.sync.dma_start(wq_sb[:], t["wq_t"][et])
                pq = psum.tile([P, 512], F32, tag="big", bufs=7, name="pq")
                for dc in range(DC):
                    nc.tensor.matmul(
                        pq[:, :TQ], wq_sb[:, dc, :], hq[:, dc, :],
                        start=(dc == 0), stop=(dc == DC - 1),
                    )
                rope(pq[:, :TQ], cosq[:], sinq[:], QT[:, et, :], TQ)

            # gathered K/V readback: emitted after the Q weight DMAs so these
            # sem-gated transfers don't head-of-line-block the weight queues;
            # K slots first (scores need K before AV needs V).
            for r in range(4):
                for et in range(KVH):
                    nc.sync.dma_start(KT[:, et, r * TL:(r + 1) * TL],
                                      t["kvb_out"][r, :, et, :])
            for r in range(4):
                for ch in range(KCL):
                    nc.sync.dma_start(V[:, KCL * r + ch, :].bitcast(F16),
                                      t["kvb_out"][r, :, KVH + ch, :])

        p_norm.release()  # hn/hq dead

        # ================= phase 2: attention ==============================
        n_rep = H // KVH
        with tc.tile_pool(name="p_att", bufs=1, side="left") as p_att, \
                tc.tile_pool(name="s3", bufs=1, side="left") as s3:
            # in ascending-extent block order only the first 64-query block
            # of each chunk's active range can be masked/invalid; the rest
            # is always fully visible.
            mask = p_att.tile([P, KC, BLK], BF16, tag="mask")
            for kc in range(KC):
                nc.sync.dma_start(mask[:, kc, :], t["maskb"][kc])
            attnT = p_att.tile([P, H, TQ], F16, tag="attnT")
            # two-deep software pipeline across heads: scores/exp/mask of
            # head h run while AV+denominator matmuls of head h-1 and the
            # normalize of head h-2 retire. The in-order PE then never waits
            # on the ACT exp chain or the DVE reciprocal.
            def emit_scores(h):
                g = h // n_rep
                es = p_att.tile([P, KC, TQ], BF16, tag="expS", bufs=3, name="es")
                # narrow chunks first: the wide chunks' slower exp evictions
                # then drain while the next pipeline stage's AV matmuls run.
                # Chunk pairs (2m, 2m+1) share offset/width; for kc >= 8 both
                # fit in one psum bank and take a single (cheaper) exp.
                for kc0 in (14, 12, 10, 8):
                    o = OFF[kc0]
                    w = TQ - o
                    ps = psum.tile([P, 2, 256], F32, tag="big", bufs=7,
                                   name="ps")
                    for i in (0, 1):
                        nc.tensor.matmul(
                            ps[:, i, 0:w],
                            KT[:, g, (kc0 + i) * P:(kc0 + i + 1) * P],
                            QT[:, h, o:TQ],
                            start=True, stop=True, skip_group_check=True,
                        )
                    nc.scalar.activation(es[:, kc0:kc0 + 2, o:],
                                         ps[:, :, 0:w], AF.Exp)
                    for i in (0, 1):
                        nc.vector.tensor_tensor(
                            es[:, kc0 + i, o:o + BLK],
                            es[:, kc0 + i, o:o + BLK],
                            mask[:, kc0 + i, :], OP.mult)
                for kc in reversed(range(8)):
                    o = OFF[kc]
                    ps = psum.tile([P, 512], F32, tag="big", bufs=7, name="ps")
                    nc.tensor.matmul(
                        ps[:, o:TQ], KT[:, g, kc * P:(kc + 1) * P],
                        QT[:, h, o:TQ],
                        start=True, stop=True,
                    )
                    nc.scalar.activation(es[:, kc, o:], ps[:, o:TQ], AF.Exp)
                    nc.vector.tensor_tensor(es[:, kc, o:o + BLK],
                                            es[:, kc, o:o + BLK],
                                            mask[:, kc, :], OP.mult)
                return es

            def emit_av_den(h, es):
                g = h // n_rep
                pav = psum.tile([P, 512], F32, tag="big", bufs=7, name="pav")
                pden = psum.tile([1, 512], F32, tag="small", bufs=1, name="pden")
                for kc in range(KC):
                    o = OFF[kc]
                    nc.tensor.matmul(pden[:, o:TQ], ones_bf[:], es[:, kc, o:],
                                     start=(kc == 0), stop=(kc == KC - 1),
                                     skip_group_check=True)
                for kc in range(KC):
                    o = OFF[kc]
                    nc.tensor.matmul(
                        pav[:, o:TQ], V[:, kc, g * P:(g + 1) * P],
                        es[:, kc, o:],
                        start=(kc == 0), stop=(kc == KC - 1),
                        skip_group_check=True,
                    )
                rden = misc.tile([1, TQ], F32, tag="rstd_tmp", bufs=2, name="rden")
                recip(rden[:], pden[:, :TQ])
                return pav, rden

            def finish_head(ph, ppav, prden, on_act=False):
                # gpsimd ucode broadcast: replaces the fp16-cast + K=1 PE
                # matmul + DVE psum copy, keeps rden fp32, and frees a psum
                # bank per head. Latency (~us) hides in the 2-deep pipeline.
                rdba = p_att.tile([P, TQ], F32, tag="rdba", bufs=2)
                nc.gpsimd.partition_broadcast(rdba[:], prden[:, :TQ])
                nc.vector.tensor_tensor(attnT[:, ph, :], ppav[:, :TQ], rdba[:],
                                        OP.mult)

            fin1 = None  # (h, pav, rden) awaiting bcast+normalize
            fin0 = None  # (h, es) awaiting av+den
            for h in range(H):
                es = emit_scores(h)
                if fin1 is not None:
                    finish_head(*fin1)
                    fin1 = None
                if fin0 is not None:
                    ph, pes = fin0
                    ppav, prden = emit_av_den(ph, pes)
                    fin1 = (ph, ppav, prden)
                fin0 = (h, es)
            ph, pes = fin0
            if fin1 is not None:
                finish_head(*fin1)
            ppav, prden = emit_av_den(ph, pes)
            finish_head(ph, ppav, prden, on_act=True)

            p_qkv.release()  # KT/QT/V dead

            # ============= phase 3: o-proj + residual + RMSNorm2 ===========
            p_res = ctx.enter_context(
                tc.tile_pool(name="p_res", bufs=1, side="right"))
            h2 = p_res.tile([P, DC, TQ], F32, tag="h2")
            mt = p_res.tile([P, DC, TQ], F16, tag="mt")
            var2_ps = psum.tile([1, 512], F32, tag="small", bufs=1, name="var2")
            for dt in range(DC):
                wo_sb = s3.tile([P, H, P], F16, tag="wo", bufs=3)
                nc.sync.dma_start(wo_sb[:], t["wo_t"][dt])
                po = psum.tile([P, 512], F32, tag="big", bufs=7, name="po")
                for ec in range(H):
                    nc.tensor.matmul(
                        po[:, :TQ], wo_sb[:, ec, :], attnT[:, ec, :],
                        start=(ec == 0), stop=(ec == H - 1),
                    )
                xqr = s3.tile([P, TQ], F32, tag="xq2", bufs=2)
                nc.sync.dma_start(xqr[:], t["xq"][dt * P:(dt + 1) * P, :])
                nc.vector.tensor_tensor(h2[:, dt, :], po[:, :TQ], xqr[:], OP.add)
                nc.scalar.activation(mt[:, dt, :], h2[:, dt, :], AF.Copy)
                sq = s3.tile([P, TQ], F16, tag="sq3", bufs=3)
                nc.vector.tensor_tensor(sq[:], h2[:, dt, :], h2[:, dt, :],
                                        OP.mult)
                nc.tensor.matmul(var2_ps[:, :TQ], ones16[:], sq[:],
                                 start=(dt == 0), stop=(dt == DC - 1))
            r2 = rstd_from_var(var2_ps[:, :TQ], D)
            rdb2 = p_res.tile([P, TQ], F32, tag="rdb2")
            bcast(r2[:], rdb2[:])

        # ================= phase 4: MLP gate/up + silu =====================
        with tc.tile_pool(name="p_gu", bufs=1, side="left") as p_gu:
            gu = p_gu.tile([P, IT, TQ], F16, tag="gu")
            with tc.tile_pool(name="s45", bufs=1, side="left") as s4:
                s5 = s4
                for it in range(IT):
                    wg_sb = s4.tile([P, DC, P], F16, tag="wgu", bufs=4)
                    nc.sync.dma_start(wg_sb[:], t["wg_t"][it])
                    wu_sb = s4.tile([P, DC, P], F16, tag="wgu", bufs=4)
                    nc.sync.dma_start(wu_sb[:], t["wu_t"][it])
                    pg = psum.tile([P, 512], F32, tag="big", bufs=7, name="pg")
                    for dc in range(DC):
                        nc.tensor.matmul(pg[:, :TQ], wg_sb[:, dc, :], mt[:, dc, :],
                                         start=(dc == 0), stop=(dc == DC - 1))
                    pu = psum.tile([P, 512], F32, tag="big", bufs=7, name="pu")
                    for dc in range(DC):
                        nc.tensor.matmul(pu[:, :TQ], wu_sb[:, dc, :], mt[:, dc, :],
                                         start=(dc == 0), stop=(dc == DC - 1))
                    # raw gate/up are projections of the unnormalized h2;
                    # apply rstd2 at the nonlinearity (t1 = g*r):
                    # silu(g*r)*(u*r) = sig(t1)*t1*u*r.
                    t1 = s4.tile([P, TQ], F32, tag="t1", bufs=3)
                    nc.vector.tensor_tensor(t1[:], pg[:, :TQ], rdb2[:], OP.mult)
                    sg = s4.tile([P, TQ], F16, tag="sg", bufs=3)
                    nc.scalar.activation(sg[:], t1[:], AF.Sigmoid)
                    t2 = s4.tile([P, TQ], F16, tag="gg", bufs=3)
                    nc.vector.tensor_tensor(t2[:], sg[:], pu[:, :TQ], OP.mult)
                    t3 = s4.tile([P, TQ], F16, tag="t3", bufs=3)
                    nc.vector.tensor_tensor(t3[:], t1[:], t2[:], OP.mult)
                    nc.vector.tensor_tensor(gu[:, it, :], t3[:], rdb2[:], OP.mult)

                # ============= phase 5: MLP down + residual ================
                for dt in range(DC):
                    wd_sb = s5.tile([P, IT, P], F16, tag="wd", bufs=2)
                    nc.sync.dma_start(wd_sb[:], t["wd_t"][dt])
                    pd = psum.tile([P, 512], F32, tag="big", bufs=7, name="pd")
                    for ic in range(IT):
                        nc.tensor.matmul(pd[:, :TQ], wd_sb[:, ic, :], gu[:, ic, :],
                                         start=(ic == 0), stop=(ic == IT - 1))
                    outp = s5.tile([P, TQ], F32, tag="out", bufs=3)
                    nc.vector.tensor_tensor(outp[:], pd[:, :TQ], h2[:, dt, :],
                                            OP.add)
                    nc.sync.dma_start(t["outT"][dt * P:(dt + 1) * P, :], outp[:])


# --------------------------------------------------------------------------
# host-side input prep
# --------------------------------------------------------------------------

def _permute_heads(w, nheads):
    """Reorder each head's 128 rows as [even dims, odd dims] so RoPE's
    interleaved pairs become contiguous halves on-chip."""
    perm = np.concatenate([np.arange(0, P, 2), np.arange(1, P, 2)])
    return w.reshape(nheads, P, -1)[:, perm, :].reshape(nheads * P, -1)


def prep_weights(cfg, wq, wk, wv, wo, w_gate, w_up, w_down, ln1_w, ln2_w):
    D, H, KVH, I = cfg["D"], cfg["H"], cfg["KVH"], cfg["I"]
    DC, IT = D // P, I // P
    f16 = np.float16
    c = np.ascontiguousarray

    wq_p = _permute_heads(wq * ln1_w[None, :], H)
    wk_p = _permute_heads(wk * ln1_w[None, :], KVH)
    wv_f = wv * ln1_w[None, :]
    wg_f = w_gate * ln2_w[None, :]
    wu_f = w_up * ln2_w[None, :]

    out = {}
    # lhsT tile layouts: [outer_tile, partition(128), inner_seq, free(128)]
    out["wq_t"] = c(wq_p.reshape(H, P, DC, P).transpose(0, 3, 2, 1).astype(f16))
    out["wk_t"] = c(wk_p.reshape(KVH, P, DC, P).transpose(0, 3, 2, 1).astype(f16))
    out["wv_r"] = c(wv_f.T.reshape(DC, P, KVH * P).astype(f16))
    out["wo_t"] = c(wo.reshape(DC, P, H, P).transpose(0, 3, 2, 1).astype(f16))
    out["wg_t"] = c(wg_f.reshape(IT, P, DC, P).transpose(0, 3, 2, 1).astype(f16))
    out["wu_t"] = c(wu_f.reshape(IT, P, DC, P).transpose(0, 3, 2, 1).astype(f16))
    out["wd_t"] = c(w_down.reshape(DC, P, IT, P).transpose(0, 3, 2, 1).astype(f16))
    return out


def core_query_index(core):
    """Balanced causal query blocks for a core, ascending-extent order."""
    j = core % 4
    blocks = [j, 7 - j, 8 + j, 15 - j, 16 + j, 23 - j, 24 + j, 31 - j]
    return np.concatenate([np.arange(BLK * s, BLK * (s + 1)) for s in blocks])


def prep_core_inputs(cfg, core, weights, hidden_states, cos, sin, attention_mask):
    """Per-core activation slices. core -> (batch, paired query sub-blocks)."""
    TQ, TKV = cfg["TQ"], cfg["TKV"]
    n_chunk = TKV // TQ
    b = core // n_chunk
    q = core % n_chunk
    ks = slice(q * TQ, (q + 1) * TQ)       # contiguous kv slice for this core
    qidx = core_query_index(core)
    scale = 128.0 ** -0.5
    c = np.ascontiguousarray
    f32 = np.float32

    m = dict(weights)
    xT = c(hidden_states[b].T.astype(f32))
    m["xT"] = c(xT[:, ks].astype(np.float16))
    m["xq"] = c(xT[:, qidx])
    m["xq16"] = c(xT[:, qidx].astype(np.float16))
    m["cos_k"] = c(cos[b, ks, :64].T.astype(f32))
    m["sin_k"] = c(sin[b, ks, :64].T.astype(f32))
    m["cos_q"] = c(cos[b, qidx, :64].T.astype(f32) * scale)
    m["sin_q"] = c(sin[b, qidx, :64].T.astype(f32) * scale)
    # per-chunk causal mask for the one 64-query block (index kc//2 in the
    # ascending-extent order) that can be partial/invalid at chunk kc; all
    # other active blocks are fully visible by construction.
    am = attention_mask[b, 0]
    kc_n = TKV // P
    mask = np.zeros((kc_n, P, BLK), ml_dtypes.bfloat16)
    for kc in range(kc_n):
        qi = qidx[BLK * (kc // 2):BLK * (kc // 2 + 1)]
        mask[kc] = (am[np.ix_(qi, np.arange(kc * P, (kc + 1) * P))]
                    > NEG_THRESH).astype(ml_dtypes.bfloat16).T
    m["maskb"] = mask
    return m


# --------------------------------------------------------------------------
# entry point
# --------------------------------------------------------------------------

_NC_CACHE = {}


def _get_nc(cfg_key):
    if cfg_key not in _NC_CACHE:
        _NC_CACHE[cfg_key] = build_nc(FULL_CFG)
    return _NC_CACHE[cfg_key]


def kernel(hidden_states, cos, sin, attention_mask,
           wq, wk, wv, wo, w_gate, w_up, w_down, ln1_w, ln2_w):
    global LAST_EXEC_NS
    cfg = FULL_CFG
    nc = _get_nc("full")

    weights = prep_weights(
        cfg,
        np.asarray(wq, np.float32), np.asarray(wk, np.float32),
        np.asarray(wv, np.float32), np.asarray(wo, np.float32),
        np.asarray(w_gate, np.float32), np.asarray(w_up, np.float32),
        np.asarray(w_down, np.float32),
        np.asarray(ln1_w, np.float32), np.asarray(ln2_w, np.float32),
    )
    hs = np.asarray(hidden_states, np.float32)
    cos = np.asarray(cos, np.float32)
    sin = np.asarray(sin, np.float32)
    am = np.asarray(attention_mask, np.float32)

    in_maps = [prep_core_inputs(cfg, c, weights, hs, cos, sin, am)
               for c in range(8)]

    trace = bool(int(os.environ.get("KERNEL_TRACE", "0")))
    trace_cores = None
    if trace and os.environ.get("KERNEL_TRACE_ALL"):
        trace_cores = list(range(8))
    res = run_bass_kernel_spmd(
        nc, in_maps, core_ids=list(range(8)), trace=trace,
        trace_cores=trace_cores,
        tmpdir=os.environ.get("KERNEL_TRACE_DIR") or None,
    )
    LAST_EXEC_NS = res.exec_time_ns

    B, S = hs.shape[0], hs.shape[1]
    n_chunk = cfg["TKV"] // cfg["TQ"]
    out = np.empty((B, S, cfg["D"]), np.float32)
    for c in range(8):
        b = c // n_chunk
        out[b, core_query_index(c), :] = res.results[c]["outT"].T
    return out
